# revision 2
# baseline (speedup 1.0000x reference)
"""Multi-head dot-product attention (RoPE, causal) on 8 NeuronCores, v2.

Sharding: data-parallel over batch (2) x tensor-parallel over heads (16 -> 4
per core). Each core projects q/k/v for its 4 heads, runs causal attention,
and computes a partial output projection; the host sums the 4 partials per
batch element.

v2 design notes (vs baseline):
- Two fused phases. Phase 1 (per 512-row block tb): V-proj, K-proj (head-
  outer), Q-proj (head-outer) with RoPE evictions on DVE overlapping the
  next head's matmuls. Phase 2: attention items (tb, h) with the PE stream
  software-pipelined: scores of item i+1 are interleaved instruction-by-
  instruction with the denominator/AV matmuls of item i, so the PE never
  waits for the (slower) scalar-engine exp, never idles, and stays at max
  p-state. Output projection for block tb is folded into the pending stream
  after its 4 heads finish, so its matmuls act as pipeline fill.
- Projection inputs/weights are bf16 (same PE rate as f32r, half the DMA
  and SBUF); attention tensors (q/k/v/exp) stay f32r; out-proj operands
  bf16. Accuracy budget ~5e-3 vs the 2e-2 gate.
- Softmax denominator reciprocal uses the fast DVE approximation (~5x
  cheaper than the exact reciprocal seen at 3.4us/op in the baseline
  trace).
- Scores are computed transposed (ST[s, t]) at 256-granular causality:
  the top two s-tiles of each block row only compute the upper half of the
  t window.  Additive masks for the diagonal pairs come from one
  precomputed [128, 2048] pattern table, applied as single fused DVE adds.
- RoPE uses a de-interleaved head dim folded into a host-side permutation
  of Wq/Wk columns (scores are permutation-invariant).  Projection PSUM is
  evicted by ACT copies (plain + half-rotated), and the rotation math runs
  on the otherwise-idle GpSimd engine, keeping DVE free for the attention
  phase.
"""

import numpy as np

B, S, E, N, D = 2, 2048, 2048, 16, 128
HL = 4           # local heads per core (8 cores = 2 batch x 4 head groups)
ND = HL * D      # 512
NT = S // 128    # 16 row tiles
NB = S // 512    # 4 row blocks
NE = E // 128    # 16 contraction tiles
MASK_VALUE = float(-0.7 * np.finfo(np.float32).max)

_NC_CACHE = {}


def _build_module():
    import concourse.bass as bass
    import concourse.mybir as mybir
    import concourse.tile as tile
    from concourse import bacc

    f32 = mybir.dt.float32
    f32r = mybir.dt.float32r
    bf16 = mybir.dt.bfloat16
    Exp = mybir.ActivationFunctionType.Exp

    nc = bacc.Bacc("TRN2", target_bir_lowering=False, debug=False, num_devices=8)

    xq_d = nc.dram_tensor("xq_t", [E, S], bf16, kind="ExternalInput").ap()
    xkv_d = nc.dram_tensor("xkv_t", [E, S], bf16, kind="ExternalInput").ap()
    wq_d = nc.dram_tensor("wq", [E, ND], bf16, kind="ExternalInput").ap()
    wk_d = nc.dram_tensor("wk", [E, ND], bf16, kind="ExternalInput").ap()
    wv_d = nc.dram_tensor("wv", [E, ND], bf16, kind="ExternalInput").ap()
    wo_d = nc.dram_tensor("wo", [ND, E], bf16, kind="ExternalInput").ap()
    csd_d = nc.dram_tensor("csd", [128, S], f32, kind="ExternalInput").ap()
    sns_d = nc.dram_tensor("sns", [128, S], f32, kind="ExternalInput").ap()
    ones_d = nc.dram_tensor("ones", [128, 128], f32, kind="ExternalInput").ap()
    msk_d = nc.dram_tensor("msk", [128, 2048], f32, kind="ExternalInput").ap()
    out_d = nc.dram_tensor("out", [S, E], f32, kind="ExternalOutput").ap()

    def load_w_grouped(pool, dram, tag):
        """[E, ND] bf16 weights as 4 tiles [128, 4*ND] (4 e-subtiles each)."""
        ws = []
        for eg in range(4):
            w = pool.tile([128, 4 * ND], bf16, tag=f"{tag}{eg}",
                          name=f"{tag}{eg}")
            nc.gpsimd.dma_start(
                w[:].rearrange("p (e n) -> p e n", e=4),
                dram[bass.ds(512 * eg, 512), :]
                .rearrange("(e p) n -> p e n", p=128))
            ws.append(w)
        return ws

    def wslice(ws, et):
        return ws[et // 4][:, bass.ds(512 * (et % 4), 512)]

    with tile.TileContext(nc) as tc:
        with tc.tile_pool(name="qkp", bufs=1) as qk_pool, \
             tc.tile_pool(name="vp", bufs=1) as v_pool, \
             tc.tile_pool(name="cst", bufs=1) as cpool:
            qT = [qk_pool.tile([128, S], bf16, tag=f"qT{h}", name=f"qT{h}")
                  for h in range(HL)]
            kT = [qk_pool.tile([128, S], bf16, tag=f"kT{h}", name=f"kT{h}")
                  for h in range(HL)]
            vG = [v_pool.tile([128, 4 * ND], f32r, tag=f"vG{g}",
                              name=f"vG{g}") for g in range(4)]
            ones = cpool.tile([128, 128], f32r, tag="ones")
            msk = cpool.tile([128, 2048], f32, tag="msk")

            # ================= Phase 1: projections =================
            with nc.named_scope("proj"), \
                 tc.tile_pool(name="wp", bufs=1) as w_pool, \
                 tc.tile_pool(name="xp", bufs=1) as x_pool, \
                 tc.tile_pool(name="tbl", bufs=1) as tpool, \
                 tc.tile_pool(name="kqs", bufs=3) as kqs_pool, \
                 tc.tile_pool(name="rtmp", bufs=2) as rope_pool, \
                 tc.tile_pool(name="kqps", bufs=1, space="PSUM") as kq_ps, \
                 tc.tile_pool(name="vps", bufs=1, space="PSUM") as v_ps:
                csd = tpool.tile([128, S], f32, tag="csd")
                sns = tpool.tile([128, S], f32, tag="sns")
                def load_x(xkv, xq, tb, ekv=None, eq=None):
                    tbs = bass.ds(512 * tb, 512)
                    for c in range(4):
                        (ekv or nc.sync).dma_start(
                            xkv[:, bass.ds(4 * c, 4)],
                            xkv_d[bass.ds(512 * c, 512), tbs]
                            .rearrange("(e p) t -> p e t", p=128))
                    for c in range(4):
                        (eq or nc.sync).dma_start(
                            xq[:, bass.ds(4 * c, 4)],
                            xq_d[bass.ds(512 * c, 512), tbs]
                            .rearrange("(e p) t -> p e t", p=128))

                # Criticality-ordered loads: first V matmul needs wv + the
                # first xkv chunk; everything else arrives later.  The
                # first-emitted triggers fire earliest, so wv/xkv0 lead.
                wv = load_w_grouped(w_pool, wv_d, "wv")
                xkv0 = x_pool.tile([128, NE, 512], bf16, tag="xkv",
                                   name="xkv")
                xq0 = x_pool.tile([128, NE, 512], bf16, tag="xq", name="xq")
                load_x(xkv0, xq0, 0)
                wk = load_w_grouped(w_pool, wk_d, "wk")
                wq = load_w_grouped(w_pool, wq_d, "wq")
                nc.gpsimd.dma_start(csd[:], csd_d[:])
                nc.gpsimd.dma_start(sns[:], sns_d[:])
                nc.gpsimd.dma_start(ones[:], ones_d[:].bitcast(f32r))
                nc.gpsimd.dma_start(msk[:], msk_d[:])

                def rope(dst, src_ps, tb):
                    """PSUM -> SBUF evict on ACT (plain + half-rotated),
                    then the rotation math on the otherwise-idle GpSimd
                    engine; frees the PSUM bank after ~1.2us and keeps DVE
                    out of the projection phase entirely."""
                    tbs = bass.ds(512 * tb, 512)
                    kq_s = kqs_pool.tile([128, 2, 512], f32, tag="kqs",
                                         name="kqs")
                    nc.scalar.copy(kq_s[:, 0], src_ps[:])
                    nc.scalar.copy(kq_s[0:64, 1], src_ps[64:128, :])
                    nc.scalar.copy(kq_s[64:128, 1], src_ps[0:64, :])
                    tmp = rope_pool.tile([128, 512], f32, tag="tmp",
                                         name="tmp")
                    tmp2 = rope_pool.tile([128, 512], f32, tag="tmp2",
                                          name="tmp2")
                    nc.gpsimd.tensor_mul(tmp[:], kq_s[:, 1], sns[:, tbs])
                    nc.gpsimd.tensor_mul(tmp2[:], kq_s[:, 0], csd[:, tbs])
                    nc.gpsimd.tensor_add(dst[:, tbs], tmp2[:], tmp[:])

                for tb in range(NB):
                    tbs = bass.ds(512 * tb, 512)
                    # x tiles for this tb (bufs=1: WAR sems delay the DMA
                    # until the previous block's reads are done; 4 chunks
                    # each so the first V matmuls can start early).
                    if tb == 0:
                        xkv, xq = xkv0, xq0
                    else:
                        xkv = x_pool.tile([128, NE, 512], bf16, tag="xkv",
                                          name="xkv")
                        xq = x_pool.tile([128, NE, 512], bf16, tag="xq",
                                         name="xq")
                        load_x(xkv, xq, tb)

                    # ---- V projection (et-outer; vps accumulate) ----
                    vps = [v_ps.tile([128, ND], f32, tag=f"v{sv}",
                                     name=f"vps{sv}") for sv in range(4)]
                    for et in range(NE):
                        for sv in range(4):
                            nc.tensor.matmul(
                                vps[sv][:], xkv[:, et, bass.ts(sv, 128)],
                                wslice(wv, et), start=(et == 0),
                                stop=(et == NE - 1))
                    for sv in range(4):
                        nc.scalar.copy(vG[tb][:, bass.ts(sv, 512)],
                                       vps[sv][:])

                    # ---- K projection (head-outer) ----
                    for h in range(HL):
                        kq = kq_ps.tile([128, 512], f32, tag=f"kq{h}",
                                        name=f"kq{h}")
                        for et in range(NE):
                            nc.tensor.matmul(
                                kq[:], wslice(wk, et)[:, bass.ts(h, 128)],
                                xkv[:, et], start=(et == 0),
                                stop=(et == NE - 1))
                        rope(kT[h], kq[:], tb)

                    # ---- Q projection (head-outer; reuses kq banks) ----
                    for h in range(HL):
                        kq = kq_ps.tile([128, 512], f32, tag=f"kq{h}",
                                        name=f"kq{h}")
                        for et in range(NE):
                            nc.tensor.matmul(
                                kq[:], wslice(wq, et)[:, bass.ts(h, 128)],
                                xq[:, et], start=(et == 0),
                                stop=(et == NE - 1))
                        rope(qT[h], kq[:], tb)

            # ================= Phase 2: attention + out-proj =========
            with nc.named_scope("attn"), \
                 tc.tile_pool(name="wop", bufs=1) as wo_pool, \
                 tc.tile_pool(name="egp", bufs=2) as eg_pool, \
                 tc.tile_pool(name="utp", bufs=2) as ut_pool, \
                 tc.tile_pool(name="rcp", bufs=2) as rc_pool, \
                 tc.tile_pool(name="ost", bufs=3) as ost_pool, \
                 tc.tile_pool(name="sps", bufs=2, space="PSUM") as sps_pool, \
                 tc.tile_pool(name="dps", bufs=1, space="PSUM") as den_pool, \
                 tc.tile_pool(name="ups", bufs=1, space="PSUM") as up_pool, \
                 tc.tile_pool(name="ops", bufs=2, space="PSUM") as op_pool:
                wo = []
                for h in range(HL):
                    w = wo_pool.tile([128, E], bf16, tag=f"wo{h}",
                                     name=f"wo{h}")
                    nc.sync.dma_start(w[:], wo_d[bass.ts(h, 128), :])
                    wo.append(w)

                # Pending PE work units (closures) from the previous item,
                # interleaved into the next item's score matmuls.  Out-proj
                # units are held back one extra item (delayed) so their
                # uT dependency (DVE norm) has time to land.
                pending = []
                delayed = []

                def drain(k, done, target):
                    while done < min(target, k):
                        pending.pop(0)()
                        done += 1
                    return done

                uT = [None]

                def emit_item(idx, tb, h):
                    for due, units in [d for d in delayed if d[0] <= idx]:
                        pending.extend(units)
                        delayed.remove((due, units))
                    # 256-granular causality: s-tiles 0..4tb+1 need the
                    # full 512-wide t window; tiles 4tb+2 / 4tb+3 only its
                    # upper half.  The two half-tiles share one sps slot and
                    # one exp, packed at eG column A.
                    nsi = 4 * (tb + 1)
                    npair = nsi // 2          # incl. the half-pair
                    frac = 2 if idx <= 2 else 1  # gentler drain at entry
                    tbs = bass.ds(512 * tb, 512)
                    tbs_hi = bass.ds(512 * tb + 256, 256)
                    A = 512 * (4 * tb + 2)
                    eGt = eg_pool.tile([128, 16 * 512], f32r, tag="eG",
                                       name="eG")
                    msk4 = msk[:].rearrange("p (a b) -> p a b", a=4)
                    k, done = len(pending), 0
                    for j in range(npair):
                        sp = sps_pool.tile([128, 2, 512], f32, tag="sp",
                                           name="sp")
                        if j == npair - 1:  # the two 256-wide half tiles
                            sph = sp[:, :, 0:256]
                            for p2 in range(2):
                                si = 4 * tb + 2 + p2
                                nc.tensor.matmul(
                                    sp[:, p2, 0:256],
                                    kT[h][:, bass.ts(si, 128)],
                                    qT[h][:, tbs_hi], start=True, stop=True)
                            nc.vector.tensor_add(
                                sph, sph, msk4[:, 0:2, 0:256])
                            nc.scalar.activation(
                                eGt[:, bass.ds(A, 512)]
                                .rearrange("p (a b) -> p a b", a=2),
                                sph, Exp)
                        else:
                            sp_flat = sp[:].rearrange("p a b -> p (a b)")
                            for p2 in range(2):
                                si = 2 * j + p2
                                nc.tensor.matmul(
                                    sp[:, p2], kT[h][:, bass.ts(si, 128)],
                                    qT[h][:, tbs], start=True, stop=True)
                            if j == 2 * tb:  # diagonal full pair
                                nc.vector.tensor_add(
                                    sp_flat, sp_flat, msk[:, bass.ds(0, 1024)])
                            nc.scalar.activation(
                                eGt[:, bass.ts(j, 1024)], sp_flat, Exp)
                        done = drain(k, done, (k * (j + 1)) // (npair * frac))
                    if frac == 1:
                        drain(k, done, k)

                    # Queue this item's den / AV / norm work.
                    den = den_pool.tile([128, 512], f32, tag="den",
                                        name="den")
                    up = up_pool.tile([128, 512], f32, tag="up", name="up")
                    rc = rc_pool.tile([128, 512], f32, tag="rec", name="rec")
                    for si in range(4 * tb + 2):
                        pending.append(
                            lambda si=si: nc.tensor.matmul(
                                den[:], ones[:], eGt[:, bass.ts(si, 512)],
                                start=(si == 0), stop=False))
                    for hh2 in range(2):
                        pending.append(
                            lambda hh2=hh2: nc.tensor.matmul(
                                den[:, 256:512], ones[:],
                                eGt[:, bass.ds(A + 256 * hh2, 256)],
                                start=False, stop=(hh2 == 1),
                                skip_group_check=True))
                    pending.append(
                        lambda: nc.vector.reciprocal_approx_fast(rc[:],
                                                                 den[:]))
                    for si in range(4 * tb + 2):
                        g, sv = si // 4, si % 4
                        pending.append(
                            lambda si=si, g=g, sv=sv: nc.tensor.matmul(
                                up[:],
                                vG[g][:, bass.ds(512 * sv + 128 * h, 128)],
                                eGt[:, bass.ts(si, 512)],
                                start=(si == 0), stop=False))
                    for hh2 in range(2):
                        si = 4 * tb + 2 + hh2
                        g, sv = si // 4, si % 4
                        pending.append(
                            lambda hh2=hh2, g=g, sv=sv: nc.tensor.matmul(
                                up[:, 256:512],
                                vG[g][:, bass.ds(512 * sv + 128 * h, 128)],
                                eGt[:, bass.ds(A + 256 * hh2, 256)],
                                start=False, stop=(hh2 == 1),
                                skip_group_check=True))
                    if h == 0:
                        uT[0] = ut_pool.tile([128, 4 * 512], bf16, tag="uT",
                                             name="uT")
                    uTt = uT[0]
                    pending.append(
                        lambda: nc.vector.tensor_mul(
                            uTt[:, bass.ts(h, 512)], up[:], rc[:]))

                    if h == HL - 1:
                        # Out-projection for this block rides the pending
                        # stream: out[t, e] = sum_h uT_h[:, t].T @ wo_h.
                        units = []
                        for tt in range(4):
                            for ec in range(4):
                                op = op_pool.tile([128, 512], f32, tag="op",
                                                  name="op")
                                for hh in range(HL):
                                    units.append(
                                        lambda tt=tt, ec=ec, hh=hh, op=op,
                                        uTt=uTt: nc.tensor.matmul(
                                            op[:],
                                            uTt[:, bass.ds(
                                                512 * hh + 128 * tt, 128)],
                                            wo[hh][:, bass.ts(ec, 512)],
                                            start=(hh == 0),
                                            stop=(hh == HL - 1)))

                                def evict(tt=tt, ec=ec, op=op, tb=tb):
                                    ob = ost_pool.tile([128, 512], f32,
                                                       tag="ob", name="ob")
                                    if ec % 2:
                                        nc.scalar.copy(ob[:], op[:])
                                    else:
                                        nc.vector.tensor_copy(ob[:], op[:])
                                    nc.sync.dma_start(
                                        out_d[bass.ts(4 * tb + tt, 128),
                                              bass.ts(ec, 512)], ob[:])
                                units.append(evict)
                        half = len(units) // 2
                        delayed.append((idx + 2, units[:half]))
                        delayed.append((idx + 3, units[half:]))

                items = [(tb, h) for tb in (1, 0, 3, 2) for h in range(HL)]
                for idx, (tb, h) in enumerate(items):
                    emit_item(idx, tb, h)
                while delayed:
                    pending.extend(delayed.pop(0)[1])
                while pending:
                    pending.pop(0)()

    nc.compile()
    return nc


def _get_module():
    if "nc" not in _NC_CACHE:
        _NC_CACHE["nc"] = _build_module()
    return _NC_CACHE["nc"]


def _host_prep(inputs_q, inputs_kv, positions, Wq, Wk, Wv, Wo):
    """Build the 8 per-core input maps."""
    import ml_dtypes
    bf16 = ml_dtypes.bfloat16
    perm = np.concatenate([np.arange(0, D, 2), np.arange(1, D, 2)])  # de-interleave
    scale = np.float32(1.0 / np.sqrt(D))
    half = D // 2
    timescale = 10000.0 ** (2.0 * np.arange(half, dtype=np.float64) / D)
    ones = np.ones((128, 128), dtype=np.float32)
    # Mask patterns for the two diagonal s-tile PAIRS of a block row:
    # variant v masks t_local < 128v + p; pair 0 = [v0|v1], pair 1 = [v2|v3].
    p_i = np.arange(128)[:, None]
    c_i = np.arange(512)[None, :]
    mv = [np.where(c_i < 128 * v + p_i, MASK_VALUE, 0.0) for v in range(4)]
    msk = np.concatenate(mv, axis=1).astype(np.float32)  # [128, 2048]

    in_maps = []
    for c in range(8):
        b = c // 4
        h0 = (c % 4) * HL
        angle = positions[b].astype(np.float64)[None, :] / timescale[:, None]  # [64,S]
        cs = np.cos(angle).astype(np.float32)
        sn = np.sin(angle).astype(np.float32)
        csd = np.concatenate([cs, cs], axis=0)               # [128, S]
        sns = np.concatenate([-sn, sn], axis=0)              # [128, S]
        wq = (Wq[:, h0:h0 + HL, :][:, :, perm] * scale).reshape(E, ND)
        wk = Wk[:, h0:h0 + HL, :][:, :, perm].reshape(E, ND)
        wv = Wv[:, h0:h0 + HL, :].reshape(E, ND)
        wo = Wo[h0:h0 + HL].reshape(ND, E)
        in_maps.append({
            "xq_t": np.ascontiguousarray(inputs_q[b].T).astype(bf16),
            "xkv_t": np.ascontiguousarray(inputs_kv[b].T).astype(bf16),
            "wq": np.ascontiguousarray(wq).astype(bf16),
            "wk": np.ascontiguousarray(wk).astype(bf16),
            "wv": np.ascontiguousarray(wv).astype(bf16),
            "wo": np.ascontiguousarray(wo).astype(bf16),
            "csd": csd, "sns": sns, "ones": ones, "msk": msk,
        })
    return in_maps


def kernel(inputs_q, inputs_kv, positions, Wq, Wk, Wv, Wo, _trace=False,
           _trace_kwargs=None):
    from concourse import bass_utils

    nc = _get_module()
    in_maps = _host_prep(inputs_q, inputs_kv, positions, Wq, Wk, Wv, Wo)
    res = bass_utils.run_bass_kernel_spmd(
        nc, in_maps, core_ids=list(range(8)), trace=_trace,
        **(_trace_kwargs or {}))
    if _trace:
        _NC_CACHE["last_results"] = res
    parts = [res.results[c]["out"] for c in range(8)]
    out0 = parts[0] + parts[1] + parts[2] + parts[3]
    out1 = parts[4] + parts[5] + parts[6] + parts[7]
    return np.stack([out0, out1]).astype(np.float32)


# revision 3
# speedup vs baseline: 1.0264x; 1.0264x over previous
"""Multi-head dot-product attention (RoPE, causal) on 8 NeuronCores, v2.

Sharding: data-parallel over batch (2) x tensor-parallel over heads (16 -> 4
per core). Each core projects q/k/v for its 4 heads, runs causal attention,
and computes a partial output projection; the host sums the 4 partials per
batch element.

v2 design notes (vs baseline):
- Two fused phases. Phase 1 (per 512-row block tb): V-proj, K-proj (head-
  outer), Q-proj (head-outer) with RoPE evictions on DVE overlapping the
  next head's matmuls. Phase 2: attention items (tb, h) with the PE stream
  software-pipelined: scores of item i+1 are interleaved instruction-by-
  instruction with the denominator/AV matmuls of item i, so the PE never
  waits for the (slower) scalar-engine exp, never idles, and stays at max
  p-state. Output projection for block tb is folded into the pending stream
  after its 4 heads finish, so its matmuls act as pipeline fill.
- Projection inputs/weights are bf16 (same PE rate as f32r, half the DMA
  and SBUF); attention tensors (q/k/v/exp) stay f32r; out-proj operands
  bf16. Accuracy budget ~5e-3 vs the 2e-2 gate.
- Softmax denominator reciprocal uses the fast DVE approximation (~5x
  cheaper than the exact reciprocal seen at 3.4us/op in the baseline
  trace).
- Scores are computed transposed (ST[s, t]) at 256-granular causality:
  the top two s-tiles of each block row only compute the upper half of the
  t window.  Additive masks for the diagonal pairs come from one
  precomputed [128, 2048] pattern table, applied as single fused DVE adds.
- RoPE uses a de-interleaved head dim folded into a host-side permutation
  of Wq/Wk columns (scores are permutation-invariant).  Projection PSUM is
  evicted by ACT copies (plain + half-rotated), and the rotation math runs
  on the otherwise-idle GpSimd engine, keeping DVE free for the attention
  phase.
"""

import numpy as np

B, S, E, N, D = 2, 2048, 2048, 16, 128
HL = 4           # local heads per core (8 cores = 2 batch x 4 head groups)
ND = HL * D      # 512
NT = S // 128    # 16 row tiles
NB = S // 512    # 4 row blocks
NE = E // 128    # 16 contraction tiles
MASK_VALUE = float(-0.7 * np.finfo(np.float32).max)

_NC_CACHE = {}


def _build_module():
    import concourse.bass as bass
    import concourse.mybir as mybir
    import concourse.tile as tile
    from concourse import bacc

    f32 = mybir.dt.float32
    f32r = mybir.dt.float32r
    bf16 = mybir.dt.bfloat16
    Exp = mybir.ActivationFunctionType.Exp

    nc = bacc.Bacc("TRN2", target_bir_lowering=False, debug=False, num_devices=8)

    xq_d = nc.dram_tensor("xq_t", [E, S], bf16, kind="ExternalInput").ap()
    xkv_d = nc.dram_tensor("xkv_t", [E, S], bf16, kind="ExternalInput").ap()
    wq_d = nc.dram_tensor("wq", [E, ND], bf16, kind="ExternalInput").ap()
    wk_d = nc.dram_tensor("wk", [E, ND], bf16, kind="ExternalInput").ap()
    wv_d = nc.dram_tensor("wv", [E, ND], bf16, kind="ExternalInput").ap()
    wo_d = nc.dram_tensor("wo", [ND, E], bf16, kind="ExternalInput").ap()
    csd_d = nc.dram_tensor("csd", [128, S], f32, kind="ExternalInput").ap()
    sns_d = nc.dram_tensor("sns", [128, S], f32, kind="ExternalInput").ap()
    ones_d = nc.dram_tensor("ones", [128, 128], f32, kind="ExternalInput").ap()
    msk_d = nc.dram_tensor("msk", [128, 2048], f32, kind="ExternalInput").ap()
    out_d = nc.dram_tensor("out", [S, E], f32, kind="ExternalOutput").ap()

    def load_w_grouped(pool, dram, tag, eng=None):
        """[E, ND] bf16 weights as 4 tiles [128, 4*ND] (4 e-subtiles each)."""
        ws = []
        for eg in range(4):
            w = pool.tile([128, 4 * ND], bf16, tag=f"{tag}{eg}",
                          name=f"{tag}{eg}")
            (eng or nc.gpsimd).dma_start(
                w[:].rearrange("p (e n) -> p e n", e=4),
                dram[bass.ds(512 * eg, 512), :]
                .rearrange("(e p) n -> p e n", p=128))
            ws.append(w)
        return ws

    def wslice(ws, et):
        return ws[et // 4][:, bass.ds(512 * (et % 4), 512)]

    with tile.TileContext(nc) as tc:
        with tc.tile_pool(name="qkp", bufs=1) as qk_pool, \
             tc.tile_pool(name="vp", bufs=1) as v_pool, \
             tc.tile_pool(name="cst", bufs=1) as cpool:
            qT = [qk_pool.tile([128, S], bf16, tag=f"qT{h}", name=f"qT{h}")
                  for h in range(HL)]
            kT = [qk_pool.tile([128, S], bf16, tag=f"kT{h}", name=f"kT{h}")
                  for h in range(HL)]
            vG = [v_pool.tile([128, 4 * ND], f32r, tag=f"vG{g}",
                              name=f"vG{g}") for g in range(4)]
            ones = cpool.tile([128, 128], f32r, tag="ones")
            msk = cpool.tile([128, 2048], f32, tag="msk")

            # ================= Phase 1: projections =================
            with nc.named_scope("proj"), \
                 tc.tile_pool(name="wp", bufs=1) as w_pool, \
                 tc.tile_pool(name="xp", bufs=1) as x_pool, \
                 tc.tile_pool(name="tbl", bufs=1) as tpool, \
                 tc.tile_pool(name="kqs", bufs=3) as kqs_pool, \
                 tc.tile_pool(name="rtmp", bufs=2) as rope_pool, \
                 tc.tile_pool(name="kqps", bufs=1, space="PSUM") as kq_ps, \
                 tc.tile_pool(name="vps", bufs=1, space="PSUM") as v_ps:
                csd = tpool.tile([128, S], f32, tag="csd")
                sns = tpool.tile([128, S], f32, tag="sns")
                def load_x(xkv, xq, tb, ekv=None, eq=None):
                    tbs = bass.ds(512 * tb, 512)
                    for c in range(4):
                        (ekv or nc.sync).dma_start(
                            xkv[:, bass.ds(4 * c, 4)],
                            xkv_d[bass.ds(512 * c, 512), tbs]
                            .rearrange("(e p) t -> p e t", p=128))
                    for c in range(4):
                        (eq or nc.sync).dma_start(
                            xq[:, bass.ds(4 * c, 4)],
                            xq_d[bass.ds(512 * c, 512), tbs]
                            .rearrange("(e p) t -> p e t", p=128))

                # Criticality-ordered loads: first V matmul needs wv + the
                # first xkv chunk; everything else arrives later.  The
                # first-emitted triggers fire earliest, so wv/xkv0 lead.
                wv = load_w_grouped(w_pool, wv_d, "wv")
                xkv0 = x_pool.tile([128, NE, 512], bf16, tag="xkv",
                                   name="xkv")
                xq0 = x_pool.tile([128, NE, 512], bf16, tag="xq", name="xq")
                load_x(xkv0, xq0, 0)
                wk = load_w_grouped(w_pool, wk_d, "wk")
                wq = load_w_grouped(w_pool, wq_d, "wq")
                nc.gpsimd.dma_start(csd[:], csd_d[:])
                nc.gpsimd.dma_start(sns[:], sns_d[:])
                nc.gpsimd.dma_start(ones[:], ones_d[:].bitcast(f32r))
                nc.gpsimd.dma_start(msk[:], msk_d[:])

                def rope(dst, src_ps, tb):
                    """PSUM -> SBUF evict on ACT (plain + half-rotated),
                    then the rotation math on the otherwise-idle GpSimd
                    engine; frees the PSUM bank after ~1.2us and keeps DVE
                    out of the projection phase entirely."""
                    tbs = bass.ds(512 * tb, 512)
                    kq_s = kqs_pool.tile([128, 2, 512], f32, tag="kqs",
                                         name="kqs")
                    nc.scalar.copy(kq_s[:, 0], src_ps[:])
                    nc.scalar.copy(kq_s[0:64, 1], src_ps[64:128, :])
                    nc.scalar.copy(kq_s[64:128, 1], src_ps[0:64, :])
                    tmp = rope_pool.tile([128, 512], f32, tag="tmp",
                                         name="tmp")
                    tmp2 = rope_pool.tile([128, 512], f32, tag="tmp2",
                                          name="tmp2")
                    nc.gpsimd.tensor_mul(tmp[:], kq_s[:, 1], sns[:, tbs])
                    nc.gpsimd.tensor_mul(tmp2[:], kq_s[:, 0], csd[:, tbs])
                    nc.gpsimd.tensor_add(dst[:, tbs], tmp2[:], tmp[:])

                for tb in range(NB):
                    tbs = bass.ds(512 * tb, 512)
                    # x tiles for this tb (bufs=1: WAR sems delay the DMA
                    # until the previous block's reads are done; 4 chunks
                    # each so the first V matmuls can start early).
                    if tb == 0:
                        xkv, xq = xkv0, xq0
                    else:
                        xkv = x_pool.tile([128, NE, 512], bf16, tag="xkv",
                                          name="xkv")
                        xq = x_pool.tile([128, NE, 512], bf16, tag="xq",
                                         name="xq")
                        load_x(xkv, xq, tb)

                    # ---- V projection (et-outer; vps accumulate) ----
                    vps = [v_ps.tile([128, ND], f32, tag=f"v{sv}",
                                     name=f"vps{sv}") for sv in range(4)]
                    for et in range(NE):
                        for sv in range(4):
                            nc.tensor.matmul(
                                vps[sv][:], xkv[:, et, bass.ts(sv, 128)],
                                wslice(wv, et), start=(et == 0),
                                stop=(et == NE - 1))
                    for sv in range(4):
                        nc.scalar.copy(vG[tb][:, bass.ts(sv, 512)],
                                       vps[sv][:])

                    # ---- K projection (head-outer) ----
                    for h in range(HL):
                        kq = kq_ps.tile([128, 512], f32, tag=f"kq{h}",
                                        name=f"kq{h}")
                        for et in range(NE):
                            nc.tensor.matmul(
                                kq[:], wslice(wk, et)[:, bass.ts(h, 128)],
                                xkv[:, et], start=(et == 0),
                                stop=(et == NE - 1))
                        rope(kT[h], kq[:], tb)

                    # ---- Q projection (head-outer; reuses kq banks) ----
                    for h in range(HL):
                        kq = kq_ps.tile([128, 512], f32, tag=f"kq{h}",
                                        name=f"kq{h}")
                        for et in range(NE):
                            nc.tensor.matmul(
                                kq[:], wslice(wq, et)[:, bass.ts(h, 128)],
                                xq[:, et], start=(et == 0),
                                stop=(et == NE - 1))
                        rope(qT[h], kq[:], tb)

            # ================= Phase 2: attention + out-proj =========
            with nc.named_scope("attn"), \
                 tc.tile_pool(name="wop", bufs=1) as wo_pool, \
                 tc.tile_pool(name="egp", bufs=2) as eg_pool, \
                 tc.tile_pool(name="utp", bufs=2) as ut_pool, \
                 tc.tile_pool(name="rcp", bufs=2) as rc_pool, \
                 tc.tile_pool(name="ost", bufs=3) as ost_pool, \
                 tc.tile_pool(name="sps", bufs=2, space="PSUM") as sps_pool, \
                 tc.tile_pool(name="dps", bufs=1, space="PSUM") as den_pool, \
                 tc.tile_pool(name="ups", bufs=1, space="PSUM") as up_pool, \
                 tc.tile_pool(name="ops", bufs=2, space="PSUM") as op_pool:
                wo = []
                for h in range(HL):
                    w = wo_pool.tile([128, E], bf16, tag=f"wo{h}",
                                     name=f"wo{h}")
                    nc.sync.dma_start(w[:], wo_d[bass.ts(h, 128), :])
                    wo.append(w)

                # Pending PE work units (closures) from the previous item,
                # interleaved into the next item's score matmuls.  Out-proj
                # units are held back one extra item (delayed) so their
                # uT dependency (DVE norm) has time to land.
                pending = []
                delayed = []

                def drain(k, done, target):
                    while done < min(target, k):
                        pending.pop(0)()
                        done += 1
                    return done

                uT = [None]

                def emit_item(idx, tb, h):
                    for due, units in [d for d in delayed if d[0] <= idx]:
                        pending.extend(units)
                        delayed.remove((due, units))
                    # 256-granular causality: s-tiles 0..4tb+1 need the
                    # full 512-wide t window; tiles 4tb+2 / 4tb+3 only its
                    # upper half.  The two half-tiles share one sps slot and
                    # one exp, packed at eG column A.
                    nsi = 4 * (tb + 1)
                    npair = nsi // 2          # incl. the half-pair
                    frac = 2 if idx <= 2 else 1  # gentler drain at entry
                    tbs = bass.ds(512 * tb, 512)
                    tbs_hi = bass.ds(512 * tb + 256, 256)
                    A = 512 * (4 * tb + 2)
                    eGt = eg_pool.tile([128, 16 * 512], f32r, tag="eG",
                                       name="eG")
                    msk4 = msk[:].rearrange("p (a b) -> p a b", a=4)
                    k, done = len(pending), 0
                    for j in range(npair):
                        sp = sps_pool.tile([128, 2, 512], f32, tag="sp",
                                           name="sp")
                        if j == npair - 1:  # the two 256-wide half tiles
                            sph = sp[:, :, 0:256]
                            for p2 in range(2):
                                si = 4 * tb + 2 + p2
                                nc.tensor.matmul(
                                    sp[:, p2, 0:256],
                                    kT[h][:, bass.ts(si, 128)],
                                    qT[h][:, tbs_hi], start=True, stop=True)
                            nc.vector.tensor_add(
                                sph, sph, msk4[:, 0:2, 0:256])
                            nc.scalar.activation(
                                eGt[:, bass.ds(A, 512)]
                                .rearrange("p (a b) -> p a b", a=2),
                                sph, Exp)
                        else:
                            sp_flat = sp[:].rearrange("p a b -> p (a b)")
                            for p2 in range(2):
                                si = 2 * j + p2
                                nc.tensor.matmul(
                                    sp[:, p2], kT[h][:, bass.ts(si, 128)],
                                    qT[h][:, tbs], start=True, stop=True)
                            if j == 2 * tb:  # diagonal full pair
                                nc.vector.tensor_add(
                                    sp_flat, sp_flat, msk[:, bass.ds(0, 1024)])
                            if idx <= 2:  # entry: halve exp latency
                                for p2 in range(2):
                                    nc.scalar.activation(
                                        eGt[:, bass.ds(1024 * j + 512 * p2,
                                                       512)], sp[:, p2], Exp)
                            else:
                                nc.scalar.activation(
                                    eGt[:, bass.ts(j, 1024)], sp_flat, Exp)
                        done = drain(k, done, (k * (j + 1)) // (npair * frac))
                    if frac == 1:
                        drain(k, done, k)

                    # Queue this item's den / AV / norm work.
                    den = den_pool.tile([128, 512], f32, tag="den",
                                        name="den")
                    up = up_pool.tile([128, 512], f32, tag="up", name="up")
                    rc = rc_pool.tile([128, 512], f32, tag="rec", name="rec")
                    for si in range(4 * tb + 2):
                        pending.append(
                            lambda si=si: nc.tensor.matmul(
                                den[:], ones[:], eGt[:, bass.ts(si, 512)],
                                start=(si == 0), stop=False))
                    for hh2 in range(2):
                        pending.append(
                            lambda hh2=hh2: nc.tensor.matmul(
                                den[:, 256:512], ones[:],
                                eGt[:, bass.ds(A + 256 * hh2, 256)],
                                start=False, stop=(hh2 == 1),
                                skip_group_check=True))
                    pending.append(
                        lambda: nc.vector.reciprocal_approx_fast(rc[:],
                                                                 den[:]))
                    for si in range(4 * tb + 2):
                        g, sv = si // 4, si % 4
                        pending.append(
                            lambda si=si, g=g, sv=sv: nc.tensor.matmul(
                                up[:],
                                vG[g][:, bass.ds(512 * sv + 128 * h, 128)],
                                eGt[:, bass.ts(si, 512)],
                                start=(si == 0), stop=False))
                    for hh2 in range(2):
                        si = 4 * tb + 2 + hh2
                        g, sv = si // 4, si % 4
                        pending.append(
                            lambda hh2=hh2, g=g, sv=sv: nc.tensor.matmul(
                                up[:, 256:512],
                                vG[g][:, bass.ds(512 * sv + 128 * h, 128)],
                                eGt[:, bass.ds(A + 256 * hh2, 256)],
                                start=False, stop=(hh2 == 1),
                                skip_group_check=True))
                    if h == 0:
                        uT[0] = ut_pool.tile([128, 4 * 512], bf16, tag="uT",
                                             name="uT")
                    uTt = uT[0]
                    pending.append(
                        lambda: nc.vector.tensor_mul(
                            uTt[:, bass.ts(h, 512)], up[:], rc[:]))

                    if h == HL - 1:
                        # Out-projection for this block rides the pending
                        # stream: out[t, e] = sum_h uT_h[:, t].T @ wo_h.
                        units = []
                        for tt in range(4):
                            for ec in range(4):
                                op = op_pool.tile([128, 512], f32, tag="op",
                                                  name="op")
                                for hh in range(HL):
                                    units.append(
                                        lambda tt=tt, ec=ec, hh=hh, op=op,
                                        uTt=uTt: nc.tensor.matmul(
                                            op[:],
                                            uTt[:, bass.ds(
                                                512 * hh + 128 * tt, 128)],
                                            wo[hh][:, bass.ts(ec, 512)],
                                            start=(hh == 0),
                                            stop=(hh == HL - 1)))

                                def evict(tt=tt, ec=ec, op=op, tb=tb):
                                    ob = ost_pool.tile([128, 512], f32,
                                                       tag="ob", name="ob")
                                    nc.vector.tensor_copy(ob[:], op[:])
                                    nc.sync.dma_start(
                                        out_d[bass.ts(4 * tb + tt, 128),
                                              bass.ts(ec, 512)], ob[:])
                                units.append(evict)
                        q = len(units) // 4
                        for part in range(4):
                            lo = q * part
                            hi = q * (part + 1) if part < 3 else len(units)
                            delayed.append((idx + 2 + part, units[lo:hi]))

                items = [(tb, h) for tb in (1, 0, 3, 2) for h in range(HL)]
                for idx, (tb, h) in enumerate(items):
                    emit_item(idx, tb, h)
                while delayed:
                    pending.extend(delayed.pop(0)[1])
                while pending:
                    pending.pop(0)()

    nc.compile()
    return nc


def _get_module():
    if "nc" not in _NC_CACHE:
        _NC_CACHE["nc"] = _build_module()
    return _NC_CACHE["nc"]


def _host_prep(inputs_q, inputs_kv, positions, Wq, Wk, Wv, Wo):
    """Build the 8 per-core input maps."""
    import ml_dtypes
    bf16 = ml_dtypes.bfloat16
    perm = np.concatenate([np.arange(0, D, 2), np.arange(1, D, 2)])  # de-interleave
    scale = np.float32(1.0 / np.sqrt(D))
    half = D // 2
    timescale = 10000.0 ** (2.0 * np.arange(half, dtype=np.float64) / D)
    ones = np.ones((128, 128), dtype=np.float32)
    # Mask patterns for the two diagonal s-tile PAIRS of a block row:
    # variant v masks t_local < 128v + p; pair 0 = [v0|v1], pair 1 = [v2|v3].
    p_i = np.arange(128)[:, None]
    c_i = np.arange(512)[None, :]
    mv = [np.where(c_i < 128 * v + p_i, MASK_VALUE, 0.0) for v in range(4)]
    msk = np.concatenate(mv, axis=1).astype(np.float32)  # [128, 2048]

    in_maps = []
    for c in range(8):
        b = c // 4
        h0 = (c % 4) * HL
        angle = positions[b].astype(np.float64)[None, :] / timescale[:, None]  # [64,S]
        cs = np.cos(angle).astype(np.float32)
        sn = np.sin(angle).astype(np.float32)
        csd = np.concatenate([cs, cs], axis=0)               # [128, S]
        sns = np.concatenate([-sn, sn], axis=0)              # [128, S]
        wq = (Wq[:, h0:h0 + HL, :][:, :, perm] * scale).reshape(E, ND)
        wk = Wk[:, h0:h0 + HL, :][:, :, perm].reshape(E, ND)
        wv = Wv[:, h0:h0 + HL, :].reshape(E, ND)
        wo = Wo[h0:h0 + HL].reshape(ND, E)
        in_maps.append({
            "xq_t": np.ascontiguousarray(inputs_q[b].T).astype(bf16),
            "xkv_t": np.ascontiguousarray(inputs_kv[b].T).astype(bf16),
            "wq": np.ascontiguousarray(wq).astype(bf16),
            "wk": np.ascontiguousarray(wk).astype(bf16),
            "wv": np.ascontiguousarray(wv).astype(bf16),
            "wo": np.ascontiguousarray(wo).astype(bf16),
            "csd": csd, "sns": sns, "ones": ones, "msk": msk,
        })
    return in_maps


def kernel(inputs_q, inputs_kv, positions, Wq, Wk, Wv, Wo, _trace=False,
           _trace_kwargs=None):
    from concourse import bass_utils

    nc = _get_module()
    in_maps = _host_prep(inputs_q, inputs_kv, positions, Wq, Wk, Wv, Wo)
    res = bass_utils.run_bass_kernel_spmd(
        nc, in_maps, core_ids=list(range(8)), trace=_trace,
        **(_trace_kwargs or {}))
    if _trace:
        _NC_CACHE["last_results"] = res
    parts = [res.results[c]["out"] for c in range(8)]
    out0 = parts[0] + parts[1] + parts[2] + parts[3]
    out1 = parts[4] + parts[5] + parts[6] + parts[7]
    return np.stack([out0, out1]).astype(np.float32)


# revision 4
# speedup vs baseline: 1.0398x; 1.0131x over previous
"""Multi-head dot-product attention (RoPE, causal) on 8 NeuronCores, v2.

Sharding: data-parallel over batch (2) x tensor-parallel over heads (16 -> 4
per core). Each core projects q/k/v for its 4 heads, runs causal attention,
and computes a partial output projection; the host sums the 4 partials per
batch element.

v2 design notes (vs baseline):
- Two fused phases. Phase 1 (per 512-row block tb): V-proj, K-proj (head-
  outer), Q-proj (head-outer) with RoPE evictions on DVE overlapping the
  next head's matmuls. Phase 2: attention items (tb, h) with the PE stream
  software-pipelined: scores of item i+1 are interleaved instruction-by-
  instruction with the denominator/AV matmuls of item i, so the PE never
  waits for the (slower) scalar-engine exp, never idles, and stays at max
  p-state. Output projection for block tb is folded into the pending stream
  after its 4 heads finish, so its matmuls act as pipeline fill.
- Projection inputs/weights are bf16 (same PE rate as f32r, half the DMA
  and SBUF); attention tensors (q/k/v/exp) stay f32r; out-proj operands
  bf16. Accuracy budget ~5e-3 vs the 2e-2 gate.
- Softmax denominator reciprocal uses the fast DVE approximation (~5x
  cheaper than the exact reciprocal seen at 3.4us/op in the baseline
  trace).
- Scores are computed transposed (ST[s, t]) at 256-granular causality:
  the top two s-tiles of each block row only compute the upper half of the
  t window.  Additive masks for the diagonal pairs come from one
  precomputed [128, 2048] pattern table, applied as single fused DVE adds.
- RoPE uses a de-interleaved head dim folded into a host-side permutation
  of Wq/Wk columns (scores are permutation-invariant).  Projection PSUM is
  evicted by ACT copies (plain + half-rotated), and the rotation math runs
  on the otherwise-idle GpSimd engine, keeping DVE free for the attention
  phase.
"""

import numpy as np

B, S, E, N, D = 2, 2048, 2048, 16, 128
HL = 4           # local heads per core (8 cores = 2 batch x 4 head groups)
ND = HL * D      # 512
NT = S // 128    # 16 row tiles
NB = S // 512    # 4 row blocks
NE = E // 128    # 16 contraction tiles
MASK_VALUE = float(-0.7 * np.finfo(np.float32).max)

_NC_CACHE = {}


def _build_module():
    import concourse.bass as bass
    import concourse.mybir as mybir
    import concourse.tile as tile
    from concourse import bacc

    f32 = mybir.dt.float32
    f32r = mybir.dt.float32r
    bf16 = mybir.dt.bfloat16
    Exp = mybir.ActivationFunctionType.Exp

    nc = bacc.Bacc("TRN2", target_bir_lowering=False, debug=False, num_devices=8)

    xq_d = nc.dram_tensor("xq_t", [E, S], bf16, kind="ExternalInput").ap()
    xkv_d = nc.dram_tensor("xkv_t", [E, S], bf16, kind="ExternalInput").ap()
    wq_d = nc.dram_tensor("wq", [E, ND], bf16, kind="ExternalInput").ap()
    wk_d = nc.dram_tensor("wk", [E, ND], bf16, kind="ExternalInput").ap()
    wv_d = nc.dram_tensor("wv", [E, ND], bf16, kind="ExternalInput").ap()
    wo_d = nc.dram_tensor("wo", [ND, E], bf16, kind="ExternalInput").ap()
    csd_d = nc.dram_tensor("csd", [128, S], f32, kind="ExternalInput").ap()
    sns_d = nc.dram_tensor("sns", [128, S], f32, kind="ExternalInput").ap()
    ones_d = nc.dram_tensor("ones", [128, 128], bf16, kind="ExternalInput").ap()
    msk_d = nc.dram_tensor("msk", [128, 2048], f32, kind="ExternalInput").ap()
    out_d = nc.dram_tensor("out", [S, E], f32, kind="ExternalOutput").ap()

    def load_w_grouped(pool, dram, tag, eng=None):
        """[E, ND] bf16 weights as 4 tiles [128, 4*ND] (4 e-subtiles each)."""
        ws = []
        for eg in range(4):
            w = pool.tile([128, 4 * ND], bf16, tag=f"{tag}{eg}",
                          name=f"{tag}{eg}")
            (eng or nc.gpsimd).dma_start(
                w[:].rearrange("p (e n) -> p e n", e=4),
                dram[bass.ds(512 * eg, 512), :]
                .rearrange("(e p) n -> p e n", p=128))
            ws.append(w)
        return ws

    def wslice(ws, et):
        return ws[et // 4][:, bass.ds(512 * (et % 4), 512)]

    with tile.TileContext(nc) as tc:
        with tc.tile_pool(name="qkp", bufs=1) as qk_pool, \
             tc.tile_pool(name="vp", bufs=1) as v_pool, \
             tc.tile_pool(name="cst", bufs=1) as cpool:
            qT = [qk_pool.tile([128, S], bf16, tag=f"qT{h}", name=f"qT{h}")
                  for h in range(HL)]
            kT = [qk_pool.tile([128, S], bf16, tag=f"kT{h}", name=f"kT{h}")
                  for h in range(HL)]
            vG = [v_pool.tile([128, 4 * ND], bf16, tag=f"vG{g}",
                              name=f"vG{g}") for g in range(4)]
            ones = cpool.tile([128, 128], bf16, tag="ones")
            msk = cpool.tile([128, 2048], f32, tag="msk")

            # ================= Phase 1: projections =================
            with nc.named_scope("proj"), \
                 tc.tile_pool(name="wp", bufs=1) as w_pool, \
                 tc.tile_pool(name="xp", bufs=1) as x_pool, \
                 tc.tile_pool(name="tbl", bufs=1) as tpool, \
                 tc.tile_pool(name="kqs", bufs=3) as kqs_pool, \
                 tc.tile_pool(name="rtmp", bufs=2) as rope_pool, \
                 tc.tile_pool(name="kqps", bufs=1, space="PSUM") as kq_ps, \
                 tc.tile_pool(name="vps", bufs=1, space="PSUM") as v_ps:
                csd = tpool.tile([128, S], f32, tag="csd")
                sns = tpool.tile([128, S], f32, tag="sns")
                def load_x(xkv, xq, tb, split=False):
                    tbs = bass.ds(512 * tb, 512)
                    for c in range(4):
                        eng = nc.gpsimd if (split and c >= 2) else nc.sync
                        eng.dma_start(
                            xkv[:, bass.ds(4 * c, 4)],
                            xkv_d[bass.ds(512 * c, 512), tbs]
                            .rearrange("(e p) t -> p e t", p=128))
                    for c in range(4):
                        eng = nc.gpsimd if (split and c >= 2) else nc.sync
                        eng.dma_start(
                            xq[:, bass.ds(4 * c, 4)],
                            xq_d[bass.ds(512 * c, 512), tbs]
                            .rearrange("(e p) t -> p e t", p=128))

                # Criticality-ordered loads: first V matmul needs wv + the
                # first xkv chunk; everything else arrives later.  The
                # first-emitted triggers fire earliest, so wv/xkv0 lead.
                wv = load_w_grouped(w_pool, wv_d, "wv")
                xkv0 = x_pool.tile([128, NE, 512], bf16, tag="xkv",
                                   name="xkv")
                xq0 = x_pool.tile([128, NE, 512], bf16, tag="xq", name="xq")
                load_x(xkv0, xq0, 0, split=True)
                wk = load_w_grouped(w_pool, wk_d, "wk")
                wq = load_w_grouped(w_pool, wq_d, "wq")
                nc.gpsimd.dma_start(csd[:], csd_d[:])
                nc.gpsimd.dma_start(sns[:], sns_d[:])
                nc.gpsimd.dma_start(ones[:], ones_d[:])
                nc.gpsimd.dma_start(msk[:], msk_d[:])

                def rope(dst, src_ps, tb):
                    """PSUM -> SBUF evict on ACT (plain + half-rotated),
                    then the rotation math on the otherwise-idle GpSimd
                    engine; frees the PSUM bank after ~1.2us and keeps DVE
                    out of the projection phase entirely."""
                    tbs = bass.ds(512 * tb, 512)
                    kq_s = kqs_pool.tile([128, 2, 512], f32, tag="kqs",
                                         name="kqs")
                    nc.scalar.copy(kq_s[:, 0], src_ps[:])
                    nc.scalar.copy(kq_s[0:64, 1], src_ps[64:128, :])
                    nc.scalar.copy(kq_s[64:128, 1], src_ps[0:64, :])
                    tmp = rope_pool.tile([128, 512], f32, tag="tmp",
                                         name="tmp")
                    tmp2 = rope_pool.tile([128, 512], f32, tag="tmp2",
                                          name="tmp2")
                    nc.gpsimd.tensor_mul(tmp[:], kq_s[:, 1], sns[:, tbs])
                    nc.gpsimd.tensor_mul(tmp2[:], kq_s[:, 0], csd[:, tbs])
                    nc.gpsimd.tensor_add(dst[:, tbs], tmp2[:], tmp[:])

                for tb in range(NB):
                    tbs = bass.ds(512 * tb, 512)
                    # x tiles for this tb (bufs=1: WAR sems delay the DMA
                    # until the previous block's reads are done; 4 chunks
                    # each so the first V matmuls can start early).
                    if tb == 0:
                        xkv, xq = xkv0, xq0
                    else:
                        xkv = x_pool.tile([128, NE, 512], bf16, tag="xkv",
                                          name="xkv")
                        xq = x_pool.tile([128, NE, 512], bf16, tag="xq",
                                         name="xq")
                        load_x(xkv, xq, tb)

                    # ---- V projection (et-outer; vps accumulate) ----
                    vps = [v_ps.tile([128, ND], f32, tag=f"v{sv}",
                                     name=f"vps{sv}") for sv in range(4)]
                    for et in range(NE):
                        for sv in range(4):
                            nc.tensor.matmul(
                                vps[sv][:], xkv[:, et, bass.ts(sv, 128)],
                                wslice(wv, et), start=(et == 0),
                                stop=(et == NE - 1))
                    for sv in range(4):
                        nc.scalar.copy(vG[tb][:, bass.ts(sv, 512)],
                                       vps[sv][:])

                    # ---- K projection (head-outer) ----
                    for h in range(HL):
                        kq = kq_ps.tile([128, 512], f32, tag=f"kq{h}",
                                        name=f"kq{h}")
                        for et in range(NE):
                            nc.tensor.matmul(
                                kq[:], wslice(wk, et)[:, bass.ts(h, 128)],
                                xkv[:, et], start=(et == 0),
                                stop=(et == NE - 1))
                        rope(kT[h], kq[:], tb)

                    # ---- Q projection (head-outer; reuses kq banks) ----
                    for h in range(HL):
                        kq = kq_ps.tile([128, 512], f32, tag=f"kq{h}",
                                        name=f"kq{h}")
                        for et in range(NE):
                            nc.tensor.matmul(
                                kq[:], wslice(wq, et)[:, bass.ts(h, 128)],
                                xq[:, et], start=(et == 0),
                                stop=(et == NE - 1))
                        rope(qT[h], kq[:], tb)

            # ================= Phase 2: attention + out-proj =========
            with nc.named_scope("attn"), \
                 tc.tile_pool(name="wop", bufs=1) as wo_pool, \
                 tc.tile_pool(name="egp", bufs=2) as eg_pool, \
                 tc.tile_pool(name="utp", bufs=2) as ut_pool, \
                 tc.tile_pool(name="rcp", bufs=2) as rc_pool, \
                 tc.tile_pool(name="ost", bufs=3) as ost_pool, \
                 tc.tile_pool(name="sps", bufs=2, space="PSUM") as sps_pool, \
                 tc.tile_pool(name="dps", bufs=1, space="PSUM") as den_pool, \
                 tc.tile_pool(name="ups", bufs=1, space="PSUM") as up_pool, \
                 tc.tile_pool(name="ops", bufs=2, space="PSUM") as op_pool:
                wo = []
                for h in range(HL):
                    w = wo_pool.tile([128, E], bf16, tag=f"wo{h}",
                                     name=f"wo{h}")
                    nc.sync.dma_start(w[:], wo_d[bass.ts(h, 128), :])
                    wo.append(w)

                # Pending PE work units (closures) from the previous item,
                # interleaved into the next item's score matmuls.  Out-proj
                # units are held back one extra item (delayed) so their
                # uT dependency (DVE norm) has time to land.
                pending = []
                delayed = []

                def drain(k, done, target):
                    while done < min(target, k):
                        pending.pop(0)()
                        done += 1
                    return done

                uT = [None]

                def emit_item(idx, tb, h):
                    for due, units in [d for d in delayed if d[0] <= idx]:
                        pending.extend(units)
                        delayed.remove((due, units))
                    # 256-granular causality: s-tiles 0..4tb+1 need the
                    # full 512-wide t window; tiles 4tb+2 / 4tb+3 only its
                    # upper half.  The two half-tiles share one sps slot and
                    # one exp, packed at eG column A.
                    nsi = 4 * (tb + 1)
                    npair = nsi // 2          # incl. the half-pair
                    frac = 2 if idx <= 2 else 1  # gentler drain at entry
                    tbs = bass.ds(512 * tb, 512)
                    tbs_hi = bass.ds(512 * tb + 256, 256)
                    A = 512 * (4 * tb + 2)
                    eGt = eg_pool.tile([128, 16 * 512], bf16, tag="eG",
                                       name="eG")
                    msk4 = msk[:].rearrange("p (a b) -> p a b", a=4)
                    k, done = len(pending), 0
                    for j in range(npair):
                        sp = sps_pool.tile([128, 2, 512], f32, tag="sp",
                                           name="sp")
                        if j == npair - 1:  # the two 256-wide half tiles
                            sph = sp[:, :, 0:256]
                            for p2 in range(2):
                                si = 4 * tb + 2 + p2
                                nc.tensor.matmul(
                                    sp[:, p2, 0:256],
                                    kT[h][:, bass.ts(si, 128)],
                                    qT[h][:, tbs_hi], start=True, stop=True)
                            nc.vector.tensor_add(
                                sph, sph, msk4[:, 0:2, 0:256])
                            nc.scalar.activation(
                                eGt[:, bass.ds(A, 512)]
                                .rearrange("p (a b) -> p a b", a=2),
                                sph, Exp)
                        else:
                            sp_flat = sp[:].rearrange("p a b -> p (a b)")
                            for p2 in range(2):
                                si = 2 * j + p2
                                nc.tensor.matmul(
                                    sp[:, p2], kT[h][:, bass.ts(si, 128)],
                                    qT[h][:, tbs], start=True, stop=True)
                            if j == 2 * tb:  # diagonal full pair
                                nc.vector.tensor_add(
                                    sp_flat, sp_flat, msk[:, bass.ds(0, 1024)])
                            if idx <= 2:  # entry: halve exp latency
                                for p2 in range(2):
                                    nc.scalar.activation(
                                        eGt[:, bass.ds(1024 * j + 512 * p2,
                                                       512)], sp[:, p2], Exp)
                            else:
                                nc.scalar.activation(
                                    eGt[:, bass.ts(j, 1024)], sp_flat, Exp)
                        done = drain(k, done, (k * (j + 1)) // (npair * frac))
                    if frac == 1:
                        drain(k, done, k)

                    # Queue this item's den / AV / norm work.
                    den = den_pool.tile([128, 512], f32, tag="den",
                                        name="den")
                    up = up_pool.tile([128, 512], f32, tag="up", name="up")
                    rc = rc_pool.tile([128, 512], f32, tag="rec", name="rec")
                    for si in range(4 * tb + 2):
                        pending.append(
                            lambda si=si: nc.tensor.matmul(
                                den[:], ones[:], eGt[:, bass.ts(si, 512)],
                                start=(si == 0), stop=False))
                    for hh2 in range(2):
                        pending.append(
                            lambda hh2=hh2: nc.tensor.matmul(
                                den[:, 256:512], ones[:],
                                eGt[:, bass.ds(A + 256 * hh2, 256)],
                                start=False, stop=(hh2 == 1),
                                skip_group_check=True))
                    pending.append(
                        lambda: nc.vector.reciprocal_approx_fast(rc[:],
                                                                 den[:]))
                    for si in range(4 * tb + 2):
                        g, sv = si // 4, si % 4
                        pending.append(
                            lambda si=si, g=g, sv=sv: nc.tensor.matmul(
                                up[:],
                                vG[g][:, bass.ds(512 * sv + 128 * h, 128)],
                                eGt[:, bass.ts(si, 512)],
                                start=(si == 0), stop=False))
                    for hh2 in range(2):
                        si = 4 * tb + 2 + hh2
                        g, sv = si // 4, si % 4
                        pending.append(
                            lambda hh2=hh2, g=g, sv=sv: nc.tensor.matmul(
                                up[:, 256:512],
                                vG[g][:, bass.ds(512 * sv + 128 * h, 128)],
                                eGt[:, bass.ds(A + 256 * hh2, 256)],
                                start=False, stop=(hh2 == 1),
                                skip_group_check=True))
                    if h == 0:
                        uT[0] = ut_pool.tile([128, 4 * 512], bf16, tag="uT",
                                             name="uT")
                    uTt = uT[0]
                    pending.append(
                        lambda: nc.vector.tensor_mul(
                            uTt[:, bass.ts(h, 512)], up[:], rc[:]))

                    if h == HL - 1:
                        # Out-projection for this block rides the pending
                        # stream: out[t, e] = sum_h uT_h[:, t].T @ wo_h.
                        units = []
                        for tt in range(4):
                            for ec in range(4):
                                op = op_pool.tile([128, 512], f32, tag="op",
                                                  name="op")
                                for hh in range(HL):
                                    units.append(
                                        lambda tt=tt, ec=ec, hh=hh, op=op,
                                        uTt=uTt: nc.tensor.matmul(
                                            op[:],
                                            uTt[:, bass.ds(
                                                512 * hh + 128 * tt, 128)],
                                            wo[hh][:, bass.ts(ec, 512)],
                                            start=(hh == 0),
                                            stop=(hh == HL - 1)))

                                def evict(tt=tt, ec=ec, op=op, tb=tb):
                                    ob = ost_pool.tile([128, 512], f32,
                                                       tag="ob", name="ob")
                                    if tb == 2 and ec % 2:
                                        nc.scalar.copy(ob[:], op[:])
                                    else:
                                        nc.vector.tensor_copy(ob[:], op[:])
                                    nc.sync.dma_start(
                                        out_d[bass.ts(4 * tb + tt, 128),
                                              bass.ts(ec, 512)], ob[:])
                                units.append(evict)
                        q = len(units) // 4
                        for part in range(4):
                            lo = q * part
                            hi = q * (part + 1) if part < 3 else len(units)
                            delayed.append((idx + 2 + part, units[lo:hi]))

                items = [(tb, h) for tb in (1, 0, 3, 2) for h in range(HL)]
                for idx, (tb, h) in enumerate(items):
                    emit_item(idx, tb, h)
                while delayed:
                    pending.extend(delayed.pop(0)[1])
                while pending:
                    pending.pop(0)()

    nc.compile()
    return nc


def _get_module():
    if "nc" not in _NC_CACHE:
        _NC_CACHE["nc"] = _build_module()
    return _NC_CACHE["nc"]


def _host_prep(inputs_q, inputs_kv, positions, Wq, Wk, Wv, Wo):
    """Build the 8 per-core input maps."""
    import ml_dtypes
    bf16 = ml_dtypes.bfloat16
    perm = np.concatenate([np.arange(0, D, 2), np.arange(1, D, 2)])  # de-interleave
    scale = np.float32(1.0 / np.sqrt(D))
    half = D // 2
    timescale = 10000.0 ** (2.0 * np.arange(half, dtype=np.float64) / D)
    ones = np.ones((128, 128), dtype=np.float32)  # cast to bf16 below
    # Mask patterns for the two diagonal s-tile PAIRS of a block row:
    # variant v masks t_local < 128v + p; pair 0 = [v0|v1], pair 1 = [v2|v3].
    p_i = np.arange(128)[:, None]
    c_i = np.arange(512)[None, :]
    mv = [np.where(c_i < 128 * v + p_i, MASK_VALUE, 0.0) for v in range(4)]
    msk = np.concatenate(mv, axis=1).astype(np.float32)  # [128, 2048]

    in_maps = []
    for c in range(8):
        b = c // 4
        h0 = (c % 4) * HL
        angle = positions[b].astype(np.float64)[None, :] / timescale[:, None]  # [64,S]
        cs = np.cos(angle).astype(np.float32)
        sn = np.sin(angle).astype(np.float32)
        csd = np.concatenate([cs, cs], axis=0)               # [128, S]
        sns = np.concatenate([-sn, sn], axis=0)              # [128, S]
        wq = (Wq[:, h0:h0 + HL, :][:, :, perm] * scale).reshape(E, ND)
        wk = Wk[:, h0:h0 + HL, :][:, :, perm].reshape(E, ND)
        wv = Wv[:, h0:h0 + HL, :].reshape(E, ND)
        wo = Wo[h0:h0 + HL].reshape(ND, E)
        in_maps.append({
            "xq_t": np.ascontiguousarray(inputs_q[b].T).astype(bf16),
            "xkv_t": np.ascontiguousarray(inputs_kv[b].T).astype(bf16),
            "wq": np.ascontiguousarray(wq).astype(bf16),
            "wk": np.ascontiguousarray(wk).astype(bf16),
            "wv": np.ascontiguousarray(wv).astype(bf16),
            "wo": np.ascontiguousarray(wo).astype(bf16),
            "csd": csd, "sns": sns, "ones": ones.astype(bf16), "msk": msk,
        })
    return in_maps


def kernel(inputs_q, inputs_kv, positions, Wq, Wk, Wv, Wo, _trace=False,
           _trace_kwargs=None):
    from concourse import bass_utils

    nc = _get_module()
    in_maps = _host_prep(inputs_q, inputs_kv, positions, Wq, Wk, Wv, Wo)
    res = bass_utils.run_bass_kernel_spmd(
        nc, in_maps, core_ids=list(range(8)), trace=_trace,
        **(_trace_kwargs or {}))
    if _trace:
        _NC_CACHE["last_results"] = res
    parts = [res.results[c]["out"] for c in range(8)]
    out0 = parts[0] + parts[1] + parts[2] + parts[3]
    out1 = parts[4] + parts[5] + parts[6] + parts[7]
    return np.stack([out0, out1]).astype(np.float32)


# revision 5
# speedup vs baseline: 1.0537x; 1.0134x over previous
"""Multi-head dot-product attention (RoPE, causal) on 8 NeuronCores, v2.

Sharding: data-parallel over batch (2) x tensor-parallel over heads (16 -> 4
per core). Each core projects q/k/v for its 4 heads, runs causal attention,
and computes a partial output projection; the host sums the 4 partials per
batch element.

v2 design notes (vs baseline):
- Two fused phases. Phase 1 (per 512-row block tb): V-proj, K-proj (head-
  outer), Q-proj (head-outer) with RoPE evictions on DVE overlapping the
  next head's matmuls. Phase 2: attention items (tb, h) with the PE stream
  software-pipelined: scores of item i+1 are interleaved instruction-by-
  instruction with the denominator/AV matmuls of item i, so the PE never
  waits for the (slower) scalar-engine exp, never idles, and stays at max
  p-state. Output projection for block tb is folded into the pending stream
  after its 4 heads finish, so its matmuls act as pipeline fill.
- Projection inputs/weights are bf16 (same PE rate as f32r, half the DMA
  and SBUF); attention tensors (q/k/v/exp) stay f32r; out-proj operands
  bf16. Accuracy budget ~5e-3 vs the 2e-2 gate.
- Softmax denominator reciprocal uses the fast DVE approximation (~5x
  cheaper than the exact reciprocal seen at 3.4us/op in the baseline
  trace).
- Scores are computed transposed (ST[s, t]) at 256-granular causality:
  the top two s-tiles of each block row only compute the upper half of the
  t window.  Additive masks for the diagonal pairs come from one
  precomputed [128, 2048] pattern table, applied as single fused DVE adds.
- RoPE uses a de-interleaved head dim folded into a host-side permutation
  of Wq/Wk columns (scores are permutation-invariant).  Projection PSUM is
  evicted by ACT copies (plain + half-rotated), and the rotation math runs
  on the otherwise-idle GpSimd engine, keeping DVE free for the attention
  phase.
"""

import numpy as np

B, S, E, N, D = 2, 2048, 2048, 16, 128
HL = 4           # local heads per core (8 cores = 2 batch x 4 head groups)
ND = HL * D      # 512
NT = S // 128    # 16 row tiles
NB = S // 512    # 4 row blocks
NE = E // 128    # 16 contraction tiles
MASK_VALUE = float(-0.7 * np.finfo(np.float32).max)

_NC_CACHE = {}


def _build_module():
    import concourse.bass as bass
    import concourse.mybir as mybir
    import concourse.tile as tile
    from concourse import bacc

    f32 = mybir.dt.float32
    f32r = mybir.dt.float32r
    bf16 = mybir.dt.bfloat16
    Exp = mybir.ActivationFunctionType.Exp

    nc = bacc.Bacc("TRN2", target_bir_lowering=False, debug=False, num_devices=8)

    xq_d = nc.dram_tensor("xq_t", [E, S], bf16, kind="ExternalInput").ap()
    xkv_d = nc.dram_tensor("xkv_t", [E, S], bf16, kind="ExternalInput").ap()
    wq_d = nc.dram_tensor("wq", [E, ND], bf16, kind="ExternalInput").ap()
    wk_d = nc.dram_tensor("wk", [E, ND], bf16, kind="ExternalInput").ap()
    wv_d = nc.dram_tensor("wv", [E, ND], bf16, kind="ExternalInput").ap()
    wo_d = nc.dram_tensor("wo", [ND, E], bf16, kind="ExternalInput").ap()
    csd_d = nc.dram_tensor("csd", [128, S], f32, kind="ExternalInput").ap()
    sns_d = nc.dram_tensor("sns", [128, S], f32, kind="ExternalInput").ap()
    ones_d = nc.dram_tensor("ones", [128, 128], bf16, kind="ExternalInput").ap()
    msk_d = nc.dram_tensor("msk", [128, 2048], f32, kind="ExternalInput").ap()
    out_d = nc.dram_tensor("out", [S, E], bf16, kind="ExternalOutput").ap()

    def load_w_grouped(pool, dram, tag, eng=None):
        """[E, ND] bf16 weights as 4 tiles [128, 4*ND] (4 e-subtiles each)."""
        ws = []
        for eg in range(4):
            w = pool.tile([128, 4 * ND], bf16, tag=f"{tag}{eg}",
                          name=f"{tag}{eg}")
            (eng or nc.gpsimd).dma_start(
                w[:].rearrange("p (e n) -> p e n", e=4),
                dram[bass.ds(512 * eg, 512), :]
                .rearrange("(e p) n -> p e n", p=128))
            ws.append(w)
        return ws

    def wslice(ws, et):
        return ws[et // 4][:, bass.ds(512 * (et % 4), 512)]

    with tile.TileContext(nc) as tc:
        with tc.tile_pool(name="qkp", bufs=1) as qk_pool, \
             tc.tile_pool(name="vp", bufs=1) as v_pool, \
             tc.tile_pool(name="cst", bufs=1) as cpool:
            qT = [qk_pool.tile([128, S], bf16, tag=f"qT{h}", name=f"qT{h}")
                  for h in range(HL)]
            kT = [qk_pool.tile([128, S], bf16, tag=f"kT{h}", name=f"kT{h}")
                  for h in range(HL)]
            vG = [v_pool.tile([128, 4 * ND], bf16, tag=f"vG{g}",
                              name=f"vG{g}") for g in range(4)]
            ones = cpool.tile([128, 128], bf16, tag="ones")
            msk = cpool.tile([128, 2048], f32, tag="msk")

            # ================= Phase 1: projections =================
            with nc.named_scope("proj"), \
                 tc.tile_pool(name="wp", bufs=1) as w_pool, \
                 tc.tile_pool(name="xp", bufs=1) as x_pool, \
                 tc.tile_pool(name="tbl", bufs=1) as tpool, \
                 tc.tile_pool(name="kqs", bufs=3) as kqs_pool, \
                 tc.tile_pool(name="rtmp", bufs=2) as rope_pool, \
                 tc.tile_pool(name="kqps", bufs=1, space="PSUM") as kq_ps, \
                 tc.tile_pool(name="vps", bufs=1, space="PSUM") as v_ps:
                csd = tpool.tile([128, S], f32, tag="csd")
                sns = tpool.tile([128, S], f32, tag="sns")
                def load_x(xkv, xq, tb, split=False):
                    tbs = bass.ds(512 * tb, 512)
                    for c in range(4):
                        eng = nc.gpsimd if (split and c >= 2) else nc.sync
                        eng.dma_start(
                            xkv[:, bass.ds(4 * c, 4)],
                            xkv_d[bass.ds(512 * c, 512), tbs]
                            .rearrange("(e p) t -> p e t", p=128))
                    for c in range(4):
                        eng = nc.gpsimd if (split and c >= 2) else nc.sync
                        eng.dma_start(
                            xq[:, bass.ds(4 * c, 4)],
                            xq_d[bass.ds(512 * c, 512), tbs]
                            .rearrange("(e p) t -> p e t", p=128))

                # Criticality-ordered loads: first V matmul needs wv + the
                # first xkv chunk; everything else arrives later.  The
                # first-emitted triggers fire earliest, so wv/xkv0 lead.
                wv = load_w_grouped(w_pool, wv_d, "wv")
                xkv0 = x_pool.tile([128, NE, 512], bf16, tag="xkv",
                                   name="xkv")
                xq0 = x_pool.tile([128, NE, 512], bf16, tag="xq", name="xq")
                load_x(xkv0, xq0, 0, split=True)
                wk = load_w_grouped(w_pool, wk_d, "wk")
                wq = load_w_grouped(w_pool, wq_d, "wq")
                nc.gpsimd.dma_start(csd[:], csd_d[:])
                nc.gpsimd.dma_start(sns[:], sns_d[:])
                nc.gpsimd.dma_start(ones[:], ones_d[:])
                nc.gpsimd.dma_start(msk[:], msk_d[:])

                def rope(dst, src_ps, tb):
                    """PSUM -> SBUF evict on ACT (plain + half-rotated),
                    then the rotation math on the otherwise-idle GpSimd
                    engine; frees the PSUM bank after ~1.2us and keeps DVE
                    out of the projection phase entirely."""
                    tbs = bass.ds(512 * tb, 512)
                    kq_s = kqs_pool.tile([128, 2, 512], f32, tag="kqs",
                                         name="kqs")
                    nc.scalar.copy(kq_s[:, 0], src_ps[:])
                    nc.scalar.copy(kq_s[0:64, 1], src_ps[64:128, :])
                    nc.scalar.copy(kq_s[64:128, 1], src_ps[0:64, :])
                    tmp = rope_pool.tile([128, 512], f32, tag="tmp",
                                         name="tmp")
                    tmp2 = rope_pool.tile([128, 512], f32, tag="tmp2",
                                          name="tmp2")
                    nc.gpsimd.tensor_mul(tmp[:], kq_s[:, 1], sns[:, tbs])
                    nc.gpsimd.tensor_mul(tmp2[:], kq_s[:, 0], csd[:, tbs])
                    nc.gpsimd.tensor_add(dst[:, tbs], tmp2[:], tmp[:])

                for tb in range(NB):
                    tbs = bass.ds(512 * tb, 512)
                    # x tiles for this tb (bufs=1: WAR sems delay the DMA
                    # until the previous block's reads are done; 4 chunks
                    # each so the first V matmuls can start early).
                    if tb == 0:
                        xkv, xq = xkv0, xq0
                    else:
                        xkv = x_pool.tile([128, NE, 512], bf16, tag="xkv",
                                          name="xkv")
                        xq = x_pool.tile([128, NE, 512], bf16, tag="xq",
                                         name="xq")
                        load_x(xkv, xq, tb)

                    # ---- V projection (et-outer; vps accumulate) ----
                    vps = [v_ps.tile([128, ND], f32, tag=f"v{sv}",
                                     name=f"vps{sv}") for sv in range(4)]
                    for et in range(NE):
                        for sv in range(4):
                            nc.tensor.matmul(
                                vps[sv][:], xkv[:, et, bass.ts(sv, 128)],
                                wslice(wv, et), start=(et == 0),
                                stop=(et == NE - 1))
                    for sv in range(4):
                        nc.scalar.copy(vG[tb][:, bass.ts(sv, 512)],
                                       vps[sv][:])

                    # ---- K projection (head-outer) ----
                    for h in range(HL):
                        kq = kq_ps.tile([128, 512], f32, tag=f"kq{h}",
                                        name=f"kq{h}")
                        for et in range(NE):
                            nc.tensor.matmul(
                                kq[:], wslice(wk, et)[:, bass.ts(h, 128)],
                                xkv[:, et], start=(et == 0),
                                stop=(et == NE - 1))
                        rope(kT[h], kq[:], tb)

                    # ---- Q projection (head-outer; reuses kq banks) ----
                    for h in range(HL):
                        kq = kq_ps.tile([128, 512], f32, tag=f"kq{h}",
                                        name=f"kq{h}")
                        for et in range(NE):
                            nc.tensor.matmul(
                                kq[:], wslice(wq, et)[:, bass.ts(h, 128)],
                                xq[:, et], start=(et == 0),
                                stop=(et == NE - 1))
                        rope(qT[h], kq[:], tb)

            # ================= Phase 2: attention + out-proj =========
            with nc.named_scope("attn"), \
                 tc.tile_pool(name="wop", bufs=1) as wo_pool, \
                 tc.tile_pool(name="egp", bufs=2) as eg_pool, \
                 tc.tile_pool(name="utp", bufs=2) as ut_pool, \
                 tc.tile_pool(name="rcp", bufs=2) as rc_pool, \
                 tc.tile_pool(name="ost", bufs=3) as ost_pool, \
                 tc.tile_pool(name="sps", bufs=2, space="PSUM") as sps_pool, \
                 tc.tile_pool(name="dps", bufs=1, space="PSUM") as den_pool, \
                 tc.tile_pool(name="ups", bufs=1, space="PSUM") as up_pool, \
                 tc.tile_pool(name="ops", bufs=2, space="PSUM") as op_pool:
                wo = []
                for h in range(HL):
                    w = wo_pool.tile([128, E], bf16, tag=f"wo{h}",
                                     name=f"wo{h}")
                    nc.sync.dma_start(w[:], wo_d[bass.ts(h, 128), :])
                    wo.append(w)

                # Pending PE work units (closures) from the previous item,
                # interleaved into the next item's score matmuls.  Out-proj
                # units are held back one extra item (delayed) so their
                # uT dependency (DVE norm) has time to land.
                pending = []
                delayed = []

                def drain(k, done, target):
                    while done < min(target, k):
                        pending.pop(0)()
                        done += 1
                    return done

                uT = [None]

                def emit_item(idx, tb, h):
                    for due, units in [d for d in delayed if d[0] <= idx]:
                        pending.extend(units)
                        delayed.remove((due, units))
                    # 256-granular causality: s-tiles 0..4tb+1 need the
                    # full 512-wide t window; tiles 4tb+2 / 4tb+3 only its
                    # upper half.  The two half-tiles share one sps slot and
                    # one exp, packed at eG column A.
                    nsi = 4 * (tb + 1)
                    npair = nsi // 2          # incl. the half-pair
                    frac = 2 if idx <= 2 else 1  # gentler drain at entry
                    tbs = bass.ds(512 * tb, 512)
                    tbs_hi = bass.ds(512 * tb + 256, 256)
                    A = 512 * (4 * tb + 2)
                    eGt = eg_pool.tile([128, 16 * 512], bf16, tag="eG",
                                       name="eG")
                    msk4 = msk[:].rearrange("p (a b) -> p a b", a=4)
                    k, done = len(pending), 0
                    for j in range(npair):
                        sp = sps_pool.tile([128, 2, 512], f32, tag="sp",
                                           name="sp")
                        if j == npair - 1:  # the two 256-wide half tiles
                            sph = sp[:, :, 0:256]
                            for p2 in range(2):
                                si = 4 * tb + 2 + p2
                                nc.tensor.matmul(
                                    sp[:, p2, 0:256],
                                    kT[h][:, bass.ts(si, 128)],
                                    qT[h][:, tbs_hi], start=True, stop=True)
                            nc.vector.tensor_add(
                                sph, sph, msk4[:, 0:2, 0:256])
                            nc.scalar.activation(
                                eGt[:, bass.ds(A, 512)]
                                .rearrange("p (a b) -> p a b", a=2),
                                sph, Exp)
                        else:
                            sp_flat = sp[:].rearrange("p a b -> p (a b)")
                            for p2 in range(2):
                                si = 2 * j + p2
                                nc.tensor.matmul(
                                    sp[:, p2], kT[h][:, bass.ts(si, 128)],
                                    qT[h][:, tbs], start=True, stop=True)
                            if j == 2 * tb:  # diagonal full pair
                                nc.vector.tensor_add(
                                    sp_flat, sp_flat, msk[:, bass.ds(0, 1024)])
                            if idx <= 2:  # entry: halve exp latency
                                for p2 in range(2):
                                    nc.scalar.activation(
                                        eGt[:, bass.ds(1024 * j + 512 * p2,
                                                       512)], sp[:, p2], Exp)
                            else:
                                nc.scalar.activation(
                                    eGt[:, bass.ts(j, 1024)], sp_flat, Exp)
                        done = drain(k, done, (k * (j + 1)) // (npair * frac))
                    if frac == 1:
                        drain(k, done, k)

                    # Queue this item's den / AV / norm work.
                    den = den_pool.tile([128, 512], f32, tag="den",
                                        name="den")
                    up = up_pool.tile([128, 512], f32, tag="up", name="up")
                    rc = rc_pool.tile([128, 512], f32, tag="rec", name="rec")
                    for si in range(4 * tb + 2):
                        pending.append(
                            lambda si=si: nc.tensor.matmul(
                                den[:], ones[:], eGt[:, bass.ts(si, 512)],
                                start=(si == 0), stop=False))
                    for hh2 in range(2):
                        pending.append(
                            lambda hh2=hh2: nc.tensor.matmul(
                                den[:, 256:512], ones[:],
                                eGt[:, bass.ds(A + 256 * hh2, 256)],
                                start=False, stop=(hh2 == 1),
                                skip_group_check=True))
                    pending.append(
                        lambda: nc.vector.reciprocal_approx_fast(rc[:],
                                                                 den[:]))
                    for si in range(4 * tb + 2):
                        g, sv = si // 4, si % 4
                        pending.append(
                            lambda si=si, g=g, sv=sv: nc.tensor.matmul(
                                up[:],
                                vG[g][:, bass.ds(512 * sv + 128 * h, 128)],
                                eGt[:, bass.ts(si, 512)],
                                start=(si == 0), stop=False))
                    for hh2 in range(2):
                        si = 4 * tb + 2 + hh2
                        g, sv = si // 4, si % 4
                        pending.append(
                            lambda hh2=hh2, g=g, sv=sv: nc.tensor.matmul(
                                up[:, 256:512],
                                vG[g][:, bass.ds(512 * sv + 128 * h, 128)],
                                eGt[:, bass.ds(A + 256 * hh2, 256)],
                                start=False, stop=(hh2 == 1),
                                skip_group_check=True))
                    if h == 0:
                        uT[0] = ut_pool.tile([128, 4 * 512], bf16, tag="uT",
                                             name="uT")
                    uTt = uT[0]
                    pending.append(
                        lambda: nc.vector.tensor_mul(
                            uTt[:, bass.ts(h, 512)], up[:], rc[:]))

                    if h == HL - 1:
                        # Out-projection for this block rides the pending
                        # stream: out[t, e] = sum_h uT_h[:, t].T @ wo_h.
                        units = []
                        for tt in range(4):
                            for ec in range(4):
                                op = op_pool.tile([128, 512], f32, tag="op",
                                                  name="op")
                                for hh in range(HL):
                                    units.append(
                                        lambda tt=tt, ec=ec, hh=hh, op=op,
                                        uTt=uTt: nc.tensor.matmul(
                                            op[:],
                                            uTt[:, bass.ds(
                                                512 * hh + 128 * tt, 128)],
                                            wo[hh][:, bass.ts(ec, 512)],
                                            start=(hh == 0),
                                            stop=(hh == HL - 1)))

                                def evict(tt=tt, ec=ec, op=op, tb=tb):
                                    ob = ost_pool.tile([128, 512], bf16,
                                                       tag="ob", name="ob")
                                    if tb == 2 and ec % 2:
                                        nc.scalar.copy(ob[:], op[:])
                                    else:
                                        nc.vector.tensor_copy(ob[:], op[:])
                                    nc.sync.dma_start(
                                        out_d[bass.ts(4 * tb + tt, 128),
                                              bass.ts(ec, 512)], ob[:])
                                units.append(evict)
                        q = len(units) // 4
                        for part in range(4):
                            lo = q * part
                            hi = q * (part + 1) if part < 3 else len(units)
                            delayed.append((idx + 2 + part, units[lo:hi]))

                items = [(tb, h) for tb in (1, 0, 3, 2) for h in range(HL)]
                for idx, (tb, h) in enumerate(items):
                    emit_item(idx, tb, h)
                while delayed:
                    pending.extend(delayed.pop(0)[1])
                while pending:
                    pending.pop(0)()

    nc.compile()
    return nc


def _get_module():
    if "nc" not in _NC_CACHE:
        _NC_CACHE["nc"] = _build_module()
    return _NC_CACHE["nc"]


def _host_prep(inputs_q, inputs_kv, positions, Wq, Wk, Wv, Wo):
    """Build the 8 per-core input maps."""
    import ml_dtypes
    bf16 = ml_dtypes.bfloat16
    perm = np.concatenate([np.arange(0, D, 2), np.arange(1, D, 2)])  # de-interleave
    scale = np.float32(1.0 / np.sqrt(D))
    half = D // 2
    timescale = 10000.0 ** (2.0 * np.arange(half, dtype=np.float64) / D)
    ones = np.ones((128, 128), dtype=np.float32)  # cast to bf16 below
    # Mask patterns for the two diagonal s-tile PAIRS of a block row:
    # variant v masks t_local < 128v + p; pair 0 = [v0|v1], pair 1 = [v2|v3].
    p_i = np.arange(128)[:, None]
    c_i = np.arange(512)[None, :]
    mv = [np.where(c_i < 128 * v + p_i, MASK_VALUE, 0.0) for v in range(4)]
    msk = np.concatenate(mv, axis=1).astype(np.float32)  # [128, 2048]

    in_maps = []
    for c in range(8):
        b = c // 4
        h0 = (c % 4) * HL
        angle = positions[b].astype(np.float64)[None, :] / timescale[:, None]  # [64,S]
        cs = np.cos(angle).astype(np.float32)
        sn = np.sin(angle).astype(np.float32)
        csd = np.concatenate([cs, cs], axis=0)               # [128, S]
        sns = np.concatenate([-sn, sn], axis=0)              # [128, S]
        wq = (Wq[:, h0:h0 + HL, :][:, :, perm] * scale).reshape(E, ND)
        wk = Wk[:, h0:h0 + HL, :][:, :, perm].reshape(E, ND)
        wv = Wv[:, h0:h0 + HL, :].reshape(E, ND)
        wo = Wo[h0:h0 + HL].reshape(ND, E)
        in_maps.append({
            "xq_t": np.ascontiguousarray(inputs_q[b].T).astype(bf16),
            "xkv_t": np.ascontiguousarray(inputs_kv[b].T).astype(bf16),
            "wq": np.ascontiguousarray(wq).astype(bf16),
            "wk": np.ascontiguousarray(wk).astype(bf16),
            "wv": np.ascontiguousarray(wv).astype(bf16),
            "wo": np.ascontiguousarray(wo).astype(bf16),
            "csd": csd, "sns": sns, "ones": ones.astype(bf16), "msk": msk,
        })
    return in_maps


def kernel(inputs_q, inputs_kv, positions, Wq, Wk, Wv, Wo, _trace=False,
           _trace_kwargs=None):
    from concourse import bass_utils

    nc = _get_module()
    in_maps = _host_prep(inputs_q, inputs_kv, positions, Wq, Wk, Wv, Wo)
    res = bass_utils.run_bass_kernel_spmd(
        nc, in_maps, core_ids=list(range(8)), trace=_trace,
        **(_trace_kwargs or {}))
    if _trace:
        _NC_CACHE["last_results"] = res
    parts = [np.asarray(res.results[c]["out"], dtype=np.float32)
             for c in range(8)]
    out0 = parts[0] + parts[1] + parts[2] + parts[3]
    out1 = parts[4] + parts[5] + parts[6] + parts[7]
    return np.stack([out0, out1]).astype(np.float32)
